# revision 26
# baseline (speedup 1.0000x reference)
"""Trainium2 Bass kernel for nn_Bert_44452911514066 (DeBERTa-style disentangled
attention BERT layer), data-parallel over batch across 8 NeuronCores.

kernel(**inputs) takes the FULL inputs (as produced by reference.setup_inputs)
and returns the FULL [S, B, H] output.

Key ideas (v4):
  - batch-DP: 2 batches per core, weights/tables replicated.
  - relative-position gather is Toeplitz: per (b,h) bucket values are expanded
    into diagonal space by matmuls, the diagonal shear applied by flat-strided
    SBUF->SBUF DMAs, and the cq tiles folded into the [k,q] score PSUM with PE
    transposes (fp32); the ck tiles enter through an identity matmul.
  - the attention loop is software-pipelined two iterations deep so the PE
    never stalls; expansion PSUM is a single 2-bank [128,640] tile so each
    eviction is one instruction.
  - softmax without max-subtraction: exp(s - 12) on ScalarE; masking and the
    denominator are folded into an augmented/masked V matrix; division on DVE.
  - token<->feature transposes (hT, ln2T) via per-tile SBUF XBAR DMAs (these
    run on the transpose DMA ring, parallel to the main ring); no DRAM
    roundtrip.  All DMAs are issued from the SP engine only (the ACT HWDGE
    queue races with cross-queue dependencies in this stack).
  - phase 4 for batch 0 is interleaved into the tail of the attention loop.
  - LayerNorm statistics alternate DVE (bn_stats) / Pool (sum & sum-of-squares)
    so neither engine serializes phase 1; normalization runs on ScalarE.
  - fp16 matmul inputs everywhere (full PE rate), fp32 accumulation.
"""
import sys
sys.path.insert(0, "/opt/trn_rl_repo")
import math
import functools
import contextlib
import numpy as np

import concourse.bass as bass
import concourse.tile as tile
from concourse import mybir
from concourse.masks import make_identity

H, NH, HD, S, B = 768, 12, 64, 512, 16
NCORES = 8
BL = B // NCORES          # batches per core
T = BL * S                # tokens per core
SCALE = 1.0 / math.sqrt(3 * HD)
EPS = 1e-7
NB = 63                   # relative buckets
WIN = 640                 # expansion window per 128-row tile
CSHIFT = 12.0             # exp shift
F16 = mybir.dt.float16
F32 = mybir.dt.float32
AF = mybir.ActivationFunctionType
OP = mybir.AluOpType

# ---------------------------------------------------------------------------
# walrus workaround: this container's walrus accepts at most ONE sync wait per
# instruction; split extra waits onto single-wait NoOps.
# ---------------------------------------------------------------------------
from concourse.vector_clock import ScopedClock

_orig_add_instruction = tile.TileContext._add_instruction


def _patched_add_instruction(self, inst):
    si = inst.sync_info
    if si is not None and si.on_wait is not None and len(si.on_wait) > 1:
        waits = list(si.on_wait)
        for i, w in enumerate(waits[:-1]):
            nop = mybir.InstNoOp(name=f"{inst.name}-wsplit{i}", ins=[], outs=[])
            nop.engine = inst.engine
            nop.sync_info = mybir.SyncInfo(on_wait=[w], on_update=[])
            _orig_add_instruction(self, nop)
        inst.sync_info = mybir.SyncInfo(
            on_wait=[waits[-1]], on_update=list(si.on_update or []))
    _orig_add_instruction(self, inst)


def _patched_drain_and_barrier(self, tick_clock, wait_clock):
    nc = self.nc
    probe = nc.sync.nop(nofuse=True)
    wait_clock.add_sem_waits(probe.ins, ScopedClock({None: tick_clock.global_clock}))
    si = probe.ins.sync_info
    waits = list(si.on_wait) if si is not None and si.on_wait else []
    if len(waits) > 1:
        probe.ins.sync_info = mybir.SyncInfo(on_wait=waits[:1], on_update=[])
        for w in waits[1:]:
            n2 = nc.sync.nop(nofuse=True)
            n2.ins.sync_info = mybir.SyncInfo(on_wait=[w], on_update=[])
    nc.sync.drain()
    nc.all_engine_barrier()
    assert self.sems is not None
    popped = nc._tile_sem_poison_stack.pop()
    assert popped is self._sem_poison
    nc.clear_and_free_semaphores(list(self.sems.allocated().values()))
    nc.all_engine_barrier()


tile.TileContext._add_instruction = _patched_add_instruction
tile.TileContext._drain_and_barrier = _patched_drain_and_barrier

MW = 1026                 # M-matrix width: 1024 shifted right by 1 (+pad)


def _shear_ap(t_ap, ncols):
    """out[p, j] = tile_flat[p*(WIN-1) + 128 + j] (per-partition offset 128-p)."""
    return bass.AP(tensor=t_ap.tensor, offset=t_ap.offset + 128,
                   ap=[[WIN - 1, 128], [1, ncols]])


# ---------------------------------------------------------------------------
# device kernel build
# ---------------------------------------------------------------------------
@functools.lru_cache(maxsize=4)
def build_module(with_bias: bool, debug: bool = False):
    nc = bass.Bass()

    hid_d = nc.dram_tensor("hid", [T, H], F32, kind="ExternalInput")
    wqkT_d = nc.dram_tensor("wqkT", [H, 2 * H], F16, kind="ExternalInput")
    wvgT_d = nc.dram_tensor("wvgT", [H, 2 * H], F16, kind="ExternalInput")
    woutT_d = nc.dram_tensor("woutT", [H, H], F16, kind="ExternalInput")
    relT_d = nc.dram_tensor("relT", [H, NB], F16, kind="ExternalInput")
    Ecq_d = nc.dram_tensor("Ecq", [NB, 1024], F16, kind="ExternalInput")
    Eck_d = nc.dram_tensor("Eck", [NB, 1024], F16, kind="ExternalInput")
    vmask_d = nc.dram_tensor("vmask", [T, 1], F32, kind="ExternalInput")
    if with_bias:
        # host-prepared: bqkc[p, f] = b_qk[128f+p] * (SCALE if f<6 else 1)
        bqkc_d = nc.dram_tensor("bqkc", [128, 12], F32, kind="ExternalInput")
        bqkr_d = nc.dram_tensor("bqkr", [1, 2 * H], F32, kind="ExternalInput")
        bvgr_d = nc.dram_tensor("bvgr", [1, 2 * H], F32, kind="ExternalInput")
        boutr_d = nc.dram_tensor("boutr", [1, H], F32, kind="ExternalInput")
    out_d = nc.dram_tensor("out", [T, H], F32, kind="ExternalOutput")

    with tile.TileContext(nc) as tc, contextlib.ExitStack() as ctx:
        persist = ctx.enter_context(tc.tile_pool(name="persist", bufs=1))
        stats = ctx.enter_context(tc.tile_pool(name="stats", bufs=6))

        # --- constants ---
        ident16 = persist.tile([128, 128], F16, tag="id16")
        make_identity(nc, ident16)
        ident32 = persist.tile([128, 128], F32, tag="id32")
        make_identity(nc, ident32)
        eps_t = persist.tile([128, 1], F32, tag="eps")
        nc.vector.memset(eps_t, EPS)
        negc_t = persist.tile([128, 1], F32, tag="negc")
        nc.vector.memset(negc_t, -CSHIFT)
        ln2T = persist.tile([128, 6, T], F16, tag="ln2T")

        # --- persistent activations/tables ---
        qk16 = persist.tile([128, 12, T], F16, tag="qk16")
        g16 = persist.tile([128, 8, H], F16, tag="g16")
        va16 = persist.tile([128, 8, NH * 65], F16, tag="va16")
        ctx16 = persist.tile([128, 8, H], F16, tag="ctx16")
        Mh = persist.tile([128, 6, MW], F16, tag="Mh")
        Mq = persist.tile([128, 6, MW], F16, tag="Mq")
        nc.vector.memset(Mh[:, :, 0:1], 0.0)
        nc.vector.memset(Mq[:, :, 0:1], 0.0)
        posp = persist.tile([64, 2 * H], F16, tag="posp")
        Ecq = persist.tile([NB, 1024], F16, tag="Ecq")
        Eck = persist.tile([NB, 1024], F16, tag="Eck")
        vmask16 = persist.tile([128, 8], F32, tag="vm")
        woutT = persist.tile([128, 6, H], F16, tag="woutT")

        # weights needed only through phase 2 live in a scoped pool
        wpool = tc.tile_pool(name="wpool", bufs=1)
        wp = wpool.__enter__()
        wqkT = wp.tile([128, 6, 2 * H], F16, tag="wqkT")
        wvgT = wp.tile([128, 6, 2 * H], F16, tag="wvgT")
        relT = wp.tile([128, 6, NB], F16, tag="relT")
        hT = wp.tile([128, 6, T], F16, tag="hT")

        # --- LayerNorm helpers ---
        def ln_finish(out16, xin, mean, var):
            rstd = stats.tile([128, 1], F32, tag="rstd")
            nc.scalar.activation(out=rstd[:], in_=var, func=AF.Sqrt,
                                 bias=eps_t[:], scale=1.0)
            nc.vector.reciprocal(out=rstd[:], in_=rstd[:])
            negmr = stats.tile([128, 1], F32, tag="negmr")
            nc.vector.scalar_tensor_tensor(
                out=negmr[:], in0=mean, scalar=-1.0, in1=rstd[:],
                op0=OP.mult, op1=OP.mult)
            nc.scalar.activation(out=out16, in_=xin, func=AF.Identity,
                                 bias=negmr[:], scale=rstd[:])

        def layernorm_dve(out16, xin):
            st = stats.tile([128, 3, 6], F32, tag="bnst")
            for sg in range(3):
                nc.vector.bn_stats(out=st[:, sg, :],
                                   in_=xin[:, 256 * sg:256 * sg + 256])
            mv = stats.tile([128, 2], F32, tag="bnmv")
            nc.vector.bn_aggr(out=mv[:], in_=st[:])
            ln_finish(out16, xin, mv[:, 0:1], mv[:, 1:2])

        def layernorm_actb(out16, xin, sqpool):
            # ACT computes E[x^2] via Square+accum; DVE sums x; Pool smalls
            sq = sqpool.tile([128, H], F32, tag="sq", bufs=2)
            s1 = stats.tile([128, 1], F32, tag="ps1")
            s2 = stats.tile([128, 1], F32, tag="ps2t")
            nc.scalar.activation(out=sq[:], in_=xin, func=AF.Square,
                                 accum_out=s2[:])
            nc.vector.tensor_reduce(out=s1[:], in_=xin,
                                    axis=mybir.AxisListType.X, op=OP.add)
            mean = stats.tile([128, 1], F32, tag="pmean")
            nc.vector.tensor_scalar_mul(out=mean[:], in0=s1[:], scalar1=1.0 / H)
            m2 = stats.tile([128, 1], F32, tag="pm2")
            nc.gpsimd.tensor_mul(m2[:], mean[:], mean[:])
            var = stats.tile([128, 1], F32, tag="pvar")
            nc.vector.scalar_tensor_tensor(
                out=var[:], in0=s2[:], scalar=1.0 / H, in1=m2[:],
                op0=OP.mult, op1=OP.subtract)
            ln_finish(out16, xin, mean[:], var[:])

        # --- phase 0 + 1: input DMAs ordered by first use; LN1; XBAR to hT.
        # Single HWDGE ring (SP) — the ACT ring has broken cross-queue
        # dependency ordering in this stack. ---
        with tc.tile_pool(name="ph1", bufs=2) as ph1:
            xts = []
            for t in range(8):
                xt = ph1.tile([128, H], F32, tag="x", name=f"x{t}", bufs=8)
                nc.sync.dma_start(out=xt[:], in_=hid_d[128 * t:128 * t + 128, :])
                xts.append(xt)
            for c in range(6):
                nc.sync.dma_start(out=wvgT[:, c, :],
                                  in_=wvgT_d[128 * c:128 * c + 128, :])
            for c in range(6):
                nc.sync.dma_start(out=wqkT[:, c, :],
                                  in_=wqkT_d[128 * c:128 * c + 128, :])
            for c in range(6):
                nc.sync.dma_start(out=relT[:, c, :],
                                  in_=relT_d[128 * c:128 * c + 128, :])
            nc.sync.dma_start(out=Ecq[:], in_=Ecq_d[:])
            nc.sync.dma_start(out=Eck[:], in_=Eck_d[:])
            nc.sync.dma_start(
                out=vmask16[:],
                in_=vmask_d[:].rearrange("(t p) one -> p (t one)", p=128))
            if with_bias:
                bqkc = persist.tile([128, 12], F32, tag="bqkc")
                nc.sync.dma_start(out=bqkc[:], in_=bqkc_d[:])
                bqkr = persist.tile([64, 2 * H], F32, tag="bqkr")
                nc.sync.dma_start(
                    out=bqkr[:],
                    in_=bass.AP(tensor=bqkr_d, offset=0, ap=[[0, 64], [1, 2 * H]]))
                bvgr = persist.tile([128, 2 * H], F32, tag="bvgr")
                nc.sync.dma_start(
                    out=bvgr[:],
                    in_=bass.AP(tensor=bvgr_d, offset=0, ap=[[0, 128], [1, 2 * H]]))
                boutr = persist.tile([128, H], F32, tag="boutr")
                nc.sync.dma_start(
                    out=boutr[:],
                    in_=bass.AP(tensor=boutr_d, offset=0, ap=[[0, 128], [1, H]]))
            for c in range(6):
                nc.sync.dma_start(out=woutT[:, c, :],
                                  in_=woutT_d[128 * c:128 * c + 128, :])

            for t in range(8):
                h16 = ph1.tile([128, H], F16, tag="h16", bufs=3)
                if t % 2 == 0:
                    layernorm_dve(h16[:], xts[t][:])
                else:
                    layernorm_actb(h16[:], xts[t][:], ph1)
                for c in range(6):
                    nc.sync.dma_start(out=hT[:, c, 128 * t:128 * t + 128],
                                      in_=h16[:, 128 * c:128 * c + 128],
                                      transpose=True)

        # --- phase 2: projections ---
        with tc.tile_pool(name="ph2ps", bufs=4, space="PSUM") as ph2ps, \
             tc.tile_pool(name="ph2", bufs=3) as ph2:
            # 2b: VG (token-major) + gelu + masked/augmented V
            for t in range(8):
                vg_t = ph2.tile([128, 2 * H], F16, tag="vg")
                for fc in range(3):
                    ps = ph2ps.tile([128, 512], F32, tag="ps2")
                    for c in range(6):
                        nc.tensor.matmul(
                            ps[:], hT[:, c, 128 * t:128 * t + 128],
                            wvgT[:, c, 512 * fc:512 * fc + 512],
                            start=(c == 0), stop=(c == 5))
                    if with_bias:
                        nc.vector.scalar_tensor_tensor(
                            out=vg_t[:, 512 * fc:512 * fc + 512], in0=ps[:], scalar=1.0,
                            in1=bvgr[:, 512 * fc:512 * fc + 512],
                            op0=OP.mult, op1=OP.add)
                    else:
                        nc.vector.tensor_copy(
                            out=vg_t[:, 512 * fc:512 * fc + 512], in_=ps[:])
                nc.scalar.activation(out=g16[:, t, :], in_=vg_t[:, H:2 * H], func=AF.Gelu)
                for hh in range(NH):
                    nc.vector.tensor_scalar_mul(
                        out=va16[:, t, 65 * hh:65 * hh + 64],
                        in0=vg_t[:, 64 * hh:64 * hh + 64],
                        scalar1=vmask16[:, t:t + 1])
                vav = va16[:, t, :].rearrange("p (h c) -> p h c", h=NH)
                nc.vector.tensor_copy(
                    out=vav[:, :, 64],
                    in_=vmask16[:, t:t + 1].to_broadcast((128, NH)))
            # 2c: QK (feature-major) — emitted last so phase 3 deps are fresh
            for f in range(12):
                for nh in range(2):
                    ps = ph2ps.tile([128, 512], F32, tag="ps2")
                    for c in range(6):
                        nc.tensor.matmul(
                            ps[:], wqkT[:, c, 128 * f:128 * f + 128],
                            hT[:, c, 512 * nh:512 * nh + 512],
                            start=(c == 0), stop=(c == 5))
                    if with_bias:
                        nc.scalar.activation(
                            out=qk16[:, f, 512 * nh:512 * nh + 512], in_=ps[:],
                            func=AF.Identity, bias=bqkc[:, f:f + 1],
                            scale=SCALE if f < 6 else 1.0)
                    else:
                        nc.scalar.activation(
                            out=qk16[:, f, 512 * nh:512 * nh + 512], in_=ps[:],
                            func=AF.Copy, bias=0.0,
                            scale=SCALE if f < 6 else 1.0)

            # 2a: pos projection (needs only wqkT/relT; runs during LN1).
            # evictions on ACT in the no-bias case (DVE is busy with LN1)
            for fc in range(3):
                ps = ph2ps.tile([128, 512], F32, tag="ps2")
                for c in range(6):
                    nc.tensor.matmul(
                        ps[:NB, :], relT[:, c, :], wqkT[:, c, 512 * fc:512 * fc + 512],
                        start=(c == 0), stop=(c == 5))
                if fc == 0:
                    segs = [(0, 512, SCALE)]
                elif fc == 1:
                    segs = [(0, 256, SCALE), (256, 512, 1.0)]
                else:
                    segs = [(0, 512, 1.0)]
                for (a, b_, sc) in segs:
                    if with_bias:
                        nc.vector.scalar_tensor_tensor(
                            out=posp[:NB, 512 * fc + a:512 * fc + b_],
                            in0=ps[:NB, a:b_], scalar=float(sc),
                            in1=bqkr[:NB, 512 * fc + a:512 * fc + b_],
                            op0=OP.mult, op1=OP.add)
                    else:
                        nc.scalar.activation(
                            out=posp[:NB, 512 * fc + a:512 * fc + b_],
                            in_=ps[:NB, a:b_], func=AF.Copy, bias=0.0,
                            scale=float(sc))
            # 2a': M matrices (hidden under the tail of LN1)
            for p in range(6):
                for half in range(2):
                    hh = 2 * p + half
                    r0 = 64 * half
                    for ec in range(2):
                        ps = ph2ps.tile([128, 512], F32, tag="ps2")
                        nc.tensor.matmul(
                            ps[r0:r0 + 64, :],
                            posp[:NB, H + 64 * hh:H + 64 * hh + 64],
                            Ecq[:, 512 * ec:512 * ec + 512],
                            start=True, stop=True, tile_position=(0, r0))
                        nc.scalar.activation(
                            out=Mh[r0:r0 + 64, p, 1 + 512 * ec:1 + 512 * ec + 512],
                            in_=ps[r0:r0 + 64, :], func=AF.Copy)
                        ps2 = ph2ps.tile([128, 512], F32, tag="ps2")
                        nc.tensor.matmul(
                            ps2[r0:r0 + 64, :],
                            posp[:NB, 64 * hh:64 * hh + 64],
                            Eck[:, 512 * ec:512 * ec + 512],
                            start=True, stop=True, tile_position=(0, r0))
                        nc.vector.tensor_copy(
                            out=Mq[r0:r0 + 64, p, 1 + 512 * ec:1 + 512 * ec + 512],
                            in_=ps2[r0:r0 + 64, :])
        wpool.__exit__(None, None, None)   # free wqkT/wvgT/relT/hT

        # --- phase 3: attention, software-pipelined over (b, head), with
        # phase 4 for batch 0 interleaved into the tail. ---
        iters = [(b, hh) for b in range(BL) for hh in range(NH)]
        NIT = len(iters)
        LOOKAHEAD = 2

        ph3ctx = contextlib.ExitStack()
        psA = ph3ctx.enter_context(tc.tile_pool(name="psA", bufs=2, space="PSUM"))
        psS = ph3ctx.enter_context(tc.tile_pool(name="psS", bufs=2, space="PSUM"))
        psP = ph3ctx.enter_context(tc.tile_pool(name="psP", bufs=2, space="PSUM"))
        shp = ph3ctx.enter_context(tc.tile_pool(name="shear", bufs=8))
        etp = ph3ctx.enter_context(tc.tile_pool(name="etp", bufs=8))
        ph4 = ph3ctx.enter_context(tc.tile_pool(name="ph4", bufs=2))

        live = {}

        def emit_A(i):
            b, hh = iters[i]
            p, half = hh // 2, hh % 2
            r0 = 64 * half
            tok0 = 512 * b
            cqsh, cksh = [], []
            for t in range(4):
                ws = 384 - 128 * t
                lq = qk16[r0:r0 + 64, p, tok0 + 128 * t:tok0 + 128 * t + 128]
                lk = qk16[r0:r0 + 64, 6 + p, tok0 + 128 * t:tok0 + 128 * t + 128]
                # single 2-bank PSUM tile per direction; one eviction each
                pq = psA.tile([128, WIN], F32, tag="pexp")
                nc.tensor.matmul(pq[:, :512], lq, Mh[r0:r0 + 64, p, ws:ws + 512],
                                 start=True, stop=True)
                nc.tensor.matmul(pq[:, 512:], lq, Mh[r0:r0 + 64, p, ws + 512:ws + 640],
                                 start=True, stop=True)
                wq = shp.tile([128, WIN], F32, tag="wqf", bufs=6)
                if t < 2:
                    nc.scalar.activation(out=wq[:], in_=pq[:], func=AF.Copy)
                else:
                    nc.vector.tensor_copy(out=wq[:], in_=pq[:])
                pk = psA.tile([128, WIN], F32, tag="pexp")
                nc.tensor.matmul(pk[:, :512], lk, Mq[r0:r0 + 64, p, ws:ws + 512],
                                 start=True, stop=True)
                nc.tensor.matmul(pk[:, 512:], lk, Mq[r0:r0 + 64, p, ws + 512:ws + 640],
                                 start=True, stop=True)
                wk = shp.tile([128, WIN], F16, tag="wkf", bufs=6)
                if t < 2:
                    nc.vector.tensor_copy(out=wk[:], in_=pk[:])
                else:
                    nc.scalar.activation(out=wk[:], in_=pk[:], func=AF.Copy)
                cq = shp.tile([128, 512], F32, tag="cqsh", name=f"cqsh{i}_{t}")
                nc.sync.dma_start(out=cq[:], in_=_shear_ap(wq[:], 512))
                cqsh.append(cq)
                ck = shp.tile([128, 512], F16, tag="cksh", name=f"cksh{i}_{t}")
                nc.sync.dma_start(out=ck[:], in_=_shear_ap(wk[:], 512))
                cksh.append(ck)
            live[i] = dict(cqsh=cqsh, cksh=cksh)

        def emit_scores(i):
            b, hh = iters[i]
            p, half = hh // 2, hh % 2
            r0 = 64 * half
            tok0 = 512 * b
            cqsh = live[i]["cqsh"]
            cksh = live[i]["cksh"]
            et = []
            for u in range(4):
                sc = psS.tile([128, 512], F32, tag="sc")
                for t in range(4):
                    nc.tensor.matmul(
                        sc[:, 128 * t:128 * t + 128],
                        cqsh[t][:, 128 * u:128 * u + 128], ident32[:],
                        is_transpose=True, start=(t == 0), stop=False)
                nc.tensor.matmul(
                    sc[:],
                    qk16[r0:r0 + 64, 6 + p, tok0 + 128 * u:tok0 + 128 * u + 128],
                    qk16[r0:r0 + 64, p, tok0:tok0 + 512],
                    start=False, stop=False)
                nc.tensor.matmul(sc[:], ident16[:], cksh[u][:],
                                 start=False, stop=True)
                e_u = etp.tile([128, 512], F16, tag="et")
                nc.scalar.activation(out=e_u[:], in_=sc[:], func=AF.Exp,
                                     bias=negc_t[:], scale=1.0)
                et.append(e_u)
            live[i]["et"] = et

        def emit_pv(i):
            b, hh = iters[i]
            et = live[i]["et"]
            for t in range(4):
                cps = psP.tile([128, 65], F32, tag="cps")
                for u in range(4):
                    nc.tensor.matmul(
                        cps[:], et[u][:, 128 * t:128 * t + 128],
                        va16[:, 4 * b + u, 65 * hh:65 * hh + 65],
                        start=(u == 0), stop=(u == 3))
                rec = stats.tile([128, 1], F32, tag="rec")
                nc.vector.reciprocal(out=rec[:], in_=cps[:, 64:65])
                nc.vector.tensor_scalar_mul(
                    out=ctx16[:, 4 * b + t, 64 * hh:64 * hh + 64],
                    in0=cps[:, 0:64], scalar1=rec[:])
            del live[i]

        def emit_ph4_tile(t):
            # gate + LN2 (Pool stats) + XBAR transposes + out projection
            cg = ph4.tile([128, H], F16, tag="cg")
            nc.gpsimd.tensor_mul(cg[:], ctx16[:, t, :], g16[:, t, :])
            ln2 = ph4.tile([128, H], F16, tag="ln2")
            layernorm_dve(ln2[:], cg[:])
            for c in range(6):
                nc.sync.dma_start(out=ln2T[:, c, 128 * t:128 * t + 128],
                                  in_=ln2[:, 128 * c:128 * c + 128],
                                  transpose=True)
            ot = ph4.tile([128, H], F32, tag="ot")
            for fc, (f0, fw) in enumerate([(0, 512), (512, 256)]):
                ps = psS.tile([128, 512], F32, tag="sc")
                for c in range(6):
                    nc.tensor.matmul(
                        ps[:, :fw], ln2T[:, c, 128 * t:128 * t + 128],
                        woutT[:, c, f0:f0 + fw],
                        start=(c == 0), stop=(c == 5))
                if with_bias:
                    nc.vector.scalar_tensor_tensor(
                        out=ot[:, f0:f0 + fw], in0=ps[:, :fw], scalar=1.0,
                        in1=boutr[:, f0:f0 + fw], op0=OP.mult, op1=OP.add)
                else:
                    nc.vector.tensor_copy(out=ot[:, f0:f0 + fw], in_=ps[:, :fw])
            nc.sync.dma_start(out=out_d[128 * t:128 * t + 128, :], in_=ot[:])

        for step in range(NIT + LOOKAHEAD):
            if step >= LOOKAHEAD:
                emit_scores(step - LOOKAHEAD)
            if step < NIT:
                emit_A(step)
            if step >= LOOKAHEAD:
                emit_pv(step - LOOKAHEAD)
        for t in range(8):
            emit_ph4_tile(t)
        ph3ctx.close()

    return nc


# ---------------------------------------------------------------------------
# host side
# ---------------------------------------------------------------------------
def _host_prep(position_indices, attention_mask):
    pi = np.asarray(position_indices)
    gvec = np.empty(1023, np.int64)
    gvec[511:] = pi[:, 0]
    gvec[:512] = pi[0, ::-1]
    d = np.arange(S)[:, None] - np.arange(S)[None, :]
    assert np.array_equal(gvec[d + 511], pi), "position_indices not Toeplitz"
    e = np.arange(1023)
    E_cq = (np.arange(NB)[:, None] == gvec[1022 - e][None, :]).astype(np.float16)
    E_ck = (np.arange(NB)[:, None] == gvec[e][None, :]).astype(np.float16)
    E_cq = np.concatenate([E_cq, np.zeros((NB, 1), np.float16)], 1)
    E_ck = np.concatenate([E_ck, np.zeros((NB, 1), np.float16)], 1)
    am = np.asarray(attention_mask).reshape(B, S)
    vmask = (~am).astype(np.float32)
    return E_cq, E_ck, vmask


def kernel(hidden_states, relative_embedding, w_qk, b_qk, w_vg, b_vg,
           w_out, b_out, attention_mask, position_indices):
    from concourse.bass_utils import run_bass_kernel_spmd

    hidden_states = np.asarray(hidden_states, dtype=np.float32)
    relative_embedding = np.asarray(relative_embedding, dtype=np.float32)
    w_qk = np.asarray(w_qk, dtype=np.float32)
    w_vg = np.asarray(w_vg, dtype=np.float32)
    w_out = np.asarray(w_out, dtype=np.float32)
    b_qk = np.asarray(b_qk, dtype=np.float32)
    b_vg = np.asarray(b_vg, dtype=np.float32)
    b_out = np.asarray(b_out, dtype=np.float32)

    with_bias = bool(np.any(b_qk) or np.any(b_vg) or np.any(b_out))
    E_cq, E_ck, vmask = _host_prep(position_indices, attention_mask)

    nc = build_module(with_bias)
    common = dict(
        wqkT=np.ascontiguousarray(w_qk.T).astype(np.float16),
        wvgT=np.ascontiguousarray(w_vg.T).astype(np.float16),
        woutT=np.ascontiguousarray(w_out.T).astype(np.float16),
        relT=np.ascontiguousarray(relative_embedding.T).astype(np.float16),
        Ecq=E_cq, Eck=E_ck)
    if with_bias:
        sc_col = np.where(np.arange(12) < 6, SCALE, 1.0).astype(np.float32)
        common["bqkc"] = np.ascontiguousarray(
            b_qk.reshape(12, 128).T * sc_col[None, :])
        sc_row = np.concatenate([np.full(H, SCALE), np.ones(H)]).astype(np.float32)
        common["bqkr"] = (b_qk * sc_row)[None, :].astype(np.float32)
        common["bvgr"] = b_vg[None, :].astype(np.float32)
        common["boutr"] = b_out[None, :].astype(np.float32)

    in_maps = []
    for core in range(NCORES):
        bsel = [BL * core + i for i in range(BL)]
        hid = np.ascontiguousarray(
            hidden_states[:, bsel, :].transpose(1, 0, 2).reshape(T, H))
        vm = np.ascontiguousarray(vmask[bsel].reshape(T, 1))
        in_maps.append(dict(common, hid=hid, vmask=vm))

    res = run_bass_kernel_spmd(nc, in_maps, list(range(NCORES)))
    out = np.empty((S, B, H), np.float32)
    for core in range(NCORES):
        o = res.results[core]["out"].reshape(BL, S, H)
        for i in range(BL):
            out[:, BL * core + i, :] = o[i]
    return out


# revision 28
# speedup vs baseline: 1.1774x; 1.1774x over previous
"""Trainium2 Bass kernel for nn_Bert_44452911514066 (DeBERTa-style disentangled
attention BERT layer), data-parallel over batch across 8 NeuronCores.

kernel(**inputs) takes the FULL inputs (as produced by reference.setup_inputs)
and returns the FULL [S, B, H] output.

Key ideas (v4):
  - batch-DP: 2 batches per core, weights/tables replicated.
  - relative-position gather is Toeplitz: per (b,h) bucket values are expanded
    into diagonal space by matmuls, the diagonal shear applied by flat-strided
    SBUF->SBUF DMAs, and the cq tiles folded into the [k,q] score PSUM with PE
    transposes (fp32); the ck tiles enter through an identity matmul.
  - the attention loop is software-pipelined two iterations deep so the PE
    never stalls; expansion PSUM is a single 2-bank [128,640] tile so each
    eviction is one instruction.
  - softmax without max-subtraction: exp(s - 12) on ScalarE; masking and the
    denominator are folded into an augmented/masked V matrix; division on DVE.
  - token<->feature transposes (hT, ln2T) via per-tile SBUF XBAR DMAs (these
    run on the transpose DMA ring, parallel to the main ring); no DRAM
    roundtrip.  All DMAs are issued from the SP engine only (the ACT HWDGE
    queue races with cross-queue dependencies in this stack).
  - phase 4 for batch 0 is interleaved into the tail of the attention loop.
  - LayerNorm statistics alternate DVE (bn_stats) / Pool (sum & sum-of-squares)
    so neither engine serializes phase 1; normalization runs on ScalarE.
  - fp16 matmul inputs everywhere (full PE rate), fp32 accumulation.
"""
import sys
sys.path.insert(0, "/opt/trn_rl_repo")
import math
import functools
import contextlib
import numpy as np

import concourse.bass as bass
import concourse.tile as tile
from concourse import mybir
from concourse.masks import make_identity

H, NH, HD, S, B = 768, 12, 64, 512, 16
NCORES = 8
BL = B // NCORES          # batches per core
T = BL * S                # tokens per core
SCALE = 1.0 / math.sqrt(3 * HD)
EPS = 1e-7
NB = 63                   # relative buckets
WIN = 640                 # expansion window per 128-row tile
CSHIFT = 12.0             # exp shift
F16 = mybir.dt.float16
F32 = mybir.dt.float32
AF = mybir.ActivationFunctionType
OP = mybir.AluOpType

# ---------------------------------------------------------------------------
# walrus workaround: this container's walrus accepts at most ONE sync wait per
# instruction; split extra waits onto single-wait NoOps.
# ---------------------------------------------------------------------------
from concourse.vector_clock import ScopedClock

_orig_add_instruction = tile.TileContext._add_instruction


def _patched_add_instruction(self, inst):
    si = inst.sync_info
    if si is not None and si.on_wait is not None and len(si.on_wait) > 1:
        waits = list(si.on_wait)
        for i, w in enumerate(waits[:-1]):
            nop = mybir.InstNoOp(name=f"{inst.name}-wsplit{i}", ins=[], outs=[])
            nop.engine = inst.engine
            nop.sync_info = mybir.SyncInfo(on_wait=[w], on_update=[])
            _orig_add_instruction(self, nop)
        inst.sync_info = mybir.SyncInfo(
            on_wait=[waits[-1]], on_update=list(si.on_update or []))
    _orig_add_instruction(self, inst)


def _patched_drain_and_barrier(self, tick_clock, wait_clock):
    nc = self.nc
    probe = nc.sync.nop(nofuse=True)
    wait_clock.add_sem_waits(probe.ins, ScopedClock({None: tick_clock.global_clock}))
    si = probe.ins.sync_info
    waits = list(si.on_wait) if si is not None and si.on_wait else []
    if len(waits) > 1:
        probe.ins.sync_info = mybir.SyncInfo(on_wait=waits[:1], on_update=[])
        for w in waits[1:]:
            n2 = nc.sync.nop(nofuse=True)
            n2.ins.sync_info = mybir.SyncInfo(on_wait=[w], on_update=[])
    nc.sync.drain()
    nc.all_engine_barrier()
    assert self.sems is not None
    popped = nc._tile_sem_poison_stack.pop()
    assert popped is self._sem_poison
    nc.clear_and_free_semaphores(list(self.sems.allocated().values()))
    nc.all_engine_barrier()


tile.TileContext._add_instruction = _patched_add_instruction
tile.TileContext._drain_and_barrier = _patched_drain_and_barrier

MW = 1026                 # M-matrix width: 1024 shifted right by 1 (+pad)


def _shear_ap(t_ap, ncols):
    """out[p, j] = tile_flat[p*(WIN-1) + 128 + j] (per-partition offset 128-p)."""
    return bass.AP(tensor=t_ap.tensor, offset=t_ap.offset + 128,
                   ap=[[WIN - 1, 128], [1, ncols]])


# ---------------------------------------------------------------------------
# device kernel build
# ---------------------------------------------------------------------------
@functools.lru_cache(maxsize=4)
def build_module(with_bias: bool, debug: bool = False):
    nc = bass.Bass()

    hid_d = nc.dram_tensor("hid", [T, H], F32, kind="ExternalInput")
    wqkT_d = nc.dram_tensor("wqkT", [H, 2 * H], F16, kind="ExternalInput")
    wvgT_d = nc.dram_tensor("wvgT", [H, 2 * H], F16, kind="ExternalInput")
    woutT_d = nc.dram_tensor("woutT", [H, H], F16, kind="ExternalInput")
    relT_d = nc.dram_tensor("relT", [H, NB], F16, kind="ExternalInput")
    Ecq_d = nc.dram_tensor("Ecq", [NB, 1024], F16, kind="ExternalInput")
    Eck_d = nc.dram_tensor("Eck", [NB, 1024], F16, kind="ExternalInput")
    vmask_d = nc.dram_tensor("vmask", [T, 1], F32, kind="ExternalInput")
    if with_bias:
        # host-prepared: bqkc[p, f] = b_qk[128f+p] * (SCALE if f<6 else 1)
        bqkc_d = nc.dram_tensor("bqkc", [128, 12], F32, kind="ExternalInput")
        bqkr_d = nc.dram_tensor("bqkr", [1, 2 * H], F32, kind="ExternalInput")
        bvgr_d = nc.dram_tensor("bvgr", [1, 2 * H], F32, kind="ExternalInput")
        boutr_d = nc.dram_tensor("boutr", [1, H], F32, kind="ExternalInput")
    out_d = nc.dram_tensor("out", [T, H], F32, kind="ExternalOutput")

    with tile.TileContext(nc) as tc, contextlib.ExitStack() as ctx:
        persist = ctx.enter_context(tc.tile_pool(name="persist", bufs=1))
        stats = ctx.enter_context(tc.tile_pool(name="stats", bufs=6))

        # --- constants ---
        ident16 = persist.tile([128, 128], F16, tag="id16")
        make_identity(nc, ident16)
        ident32 = persist.tile([128, 128], F32, tag="id32")
        make_identity(nc, ident32)
        eps_t = persist.tile([128, 1], F32, tag="eps")
        nc.vector.memset(eps_t, EPS)
        negc_t = persist.tile([128, 1], F32, tag="negc")
        nc.vector.memset(negc_t, -CSHIFT)
        ln2T = persist.tile([128, 6, T], F16, tag="ln2T")

        # --- persistent activations/tables ---
        qk16 = persist.tile([128, 12, T], F16, tag="qk16")
        g16 = persist.tile([128, 8, H], F16, tag="g16")
        va16 = persist.tile([128, 8, NH * 65], F16, tag="va16")
        ctx16 = persist.tile([128, 8, H], F16, tag="ctx16")
        Mh = persist.tile([128, 6, MW], F16, tag="Mh")
        Mq = persist.tile([128, 6, MW], F16, tag="Mq")
        nc.vector.memset(Mh[:, :, 0:1], 0.0)
        nc.vector.memset(Mq[:, :, 0:1], 0.0)
        posp = persist.tile([64, 2 * H], F16, tag="posp")
        Ecq = persist.tile([NB, 1024], F16, tag="Ecq")
        Eck = persist.tile([NB, 1024], F16, tag="Eck")
        vmask16 = persist.tile([128, 8], F32, tag="vm")
        woutT = persist.tile([128, 6, H], F16, tag="woutT")

        # weights needed only through phase 2 live in a scoped pool
        wpool = tc.tile_pool(name="wpool", bufs=1)
        wp = wpool.__enter__()
        wqkT = wp.tile([128, 6, 2 * H], F16, tag="wqkT")
        wvgT = wp.tile([128, 6, 2 * H], F16, tag="wvgT")
        relT = wp.tile([128, 6, NB], F16, tag="relT")
        hT = wp.tile([128, 6, T], F16, tag="hT")

        # --- LayerNorm helpers ---
        def ln_finish(out16, xin, mean, var):
            rstd = stats.tile([128, 1], F32, tag="rstd")
            nc.scalar.activation(out=rstd[:], in_=var, func=AF.Sqrt,
                                 bias=eps_t[:], scale=1.0)
            nc.vector.reciprocal(out=rstd[:], in_=rstd[:])
            negmr = stats.tile([128, 1], F32, tag="negmr")
            nc.vector.scalar_tensor_tensor(
                out=negmr[:], in0=mean, scalar=-1.0, in1=rstd[:],
                op0=OP.mult, op1=OP.mult)
            nc.scalar.activation(out=out16, in_=xin, func=AF.Identity,
                                 bias=negmr[:], scale=rstd[:])

        def layernorm_dve(out16, xin):
            st = stats.tile([128, 3, 6], F32, tag="bnst")
            for sg in range(3):
                nc.vector.bn_stats(out=st[:, sg, :],
                                   in_=xin[:, 256 * sg:256 * sg + 256])
            mv = stats.tile([128, 2], F32, tag="bnmv")
            nc.vector.bn_aggr(out=mv[:], in_=st[:])
            ln_finish(out16, xin, mv[:, 0:1], mv[:, 1:2])

        def layernorm_actb(out16, xin, sqpool):
            # ACT computes E[x^2] via Square+accum; DVE sums x; Pool smalls
            sq = sqpool.tile([128, H], F32, tag="sq", bufs=2)
            s1 = stats.tile([128, 1], F32, tag="ps1")
            s2 = stats.tile([128, 1], F32, tag="ps2t")
            nc.scalar.activation(out=sq[:], in_=xin, func=AF.Square,
                                 accum_out=s2[:])
            nc.vector.tensor_reduce(out=s1[:], in_=xin,
                                    axis=mybir.AxisListType.X, op=OP.add)
            mean = stats.tile([128, 1], F32, tag="pmean")
            nc.vector.tensor_scalar_mul(out=mean[:], in0=s1[:], scalar1=1.0 / H)
            m2 = stats.tile([128, 1], F32, tag="pm2")
            nc.gpsimd.tensor_mul(m2[:], mean[:], mean[:])
            var = stats.tile([128, 1], F32, tag="pvar")
            nc.vector.scalar_tensor_tensor(
                out=var[:], in0=s2[:], scalar=1.0 / H, in1=m2[:],
                op0=OP.mult, op1=OP.subtract)
            ln_finish(out16, xin, mean[:], var[:])

        # --- phase 0 + 1: input DMAs ordered by first use; LN1; XBAR to hT.
        # Single HWDGE ring (SP) — the ACT ring has broken cross-queue
        # dependency ordering in this stack. ---
        with tc.tile_pool(name="ph1", bufs=2) as ph1:
            xts = []
            for t in range(8):
                xt = ph1.tile([128, H], F32, tag="x", name=f"x{t}", bufs=8)
                nc.sync.dma_start(out=xt[:], in_=hid_d[128 * t:128 * t + 128, :])
                xts.append(xt)
            for c in range(6):
                nc.sync.dma_start(out=wvgT[:, c, :],
                                  in_=wvgT_d[128 * c:128 * c + 128, :])
            for c in range(6):
                nc.sync.dma_start(out=wqkT[:, c, :],
                                  in_=wqkT_d[128 * c:128 * c + 128, :])
            for c in range(6):
                nc.sync.dma_start(out=relT[:, c, :],
                                  in_=relT_d[128 * c:128 * c + 128, :])
            nc.sync.dma_start(out=Ecq[:], in_=Ecq_d[:])
            nc.sync.dma_start(out=Eck[:], in_=Eck_d[:])
            nc.sync.dma_start(
                out=vmask16[:],
                in_=vmask_d[:].rearrange("(t p) one -> p (t one)", p=128))
            if with_bias:
                bqkc = persist.tile([128, 12], F32, tag="bqkc")
                nc.sync.dma_start(out=bqkc[:], in_=bqkc_d[:])
                bqkr = persist.tile([64, 2 * H], F32, tag="bqkr")
                nc.sync.dma_start(
                    out=bqkr[:],
                    in_=bass.AP(tensor=bqkr_d, offset=0, ap=[[0, 64], [1, 2 * H]]))
                bvgr = persist.tile([128, 2 * H], F32, tag="bvgr")
                nc.sync.dma_start(
                    out=bvgr[:],
                    in_=bass.AP(tensor=bvgr_d, offset=0, ap=[[0, 128], [1, 2 * H]]))
                boutr = persist.tile([128, H], F32, tag="boutr")
                nc.sync.dma_start(
                    out=boutr[:],
                    in_=bass.AP(tensor=boutr_d, offset=0, ap=[[0, 128], [1, H]]))
            for c in range(6):
                nc.sync.dma_start(out=woutT[:, c, :],
                                  in_=woutT_d[128 * c:128 * c + 128, :])

            with tc.tile_pool(name="hps", bufs=3, space="PSUM") as hps:
                for t in range(8):
                    h16 = ph1.tile([128, H], F16, tag="h16", bufs=3)
                    if t % 2 == 0:
                        layernorm_dve(h16[:], xts[t][:])
                    else:
                        layernorm_actb(h16[:], xts[t][:], ph1)
                    for c in range(6):
                        tp = hps.tile([128, 128], F16, tag="tps")
                        nc.tensor.matmul(tp[:], h16[:, 128 * c:128 * c + 128],
                                         ident16[:], is_transpose=True)
                        if c % 2 == 0:
                            nc.scalar.activation(
                                out=hT[:, c, 128 * t:128 * t + 128],
                                in_=tp[:], func=AF.Copy)
                        else:
                            nc.vector.tensor_copy(
                                out=hT[:, c, 128 * t:128 * t + 128], in_=tp[:])

        # --- phase 2: projections ---
        with tc.tile_pool(name="ph2ps", bufs=4, space="PSUM") as ph2ps, \
             tc.tile_pool(name="ph2", bufs=3) as ph2:
            # 2b: VG (token-major) + gelu + masked/augmented V
            for t in range(8):
                vg_t = ph2.tile([128, 2 * H], F16, tag="vg")
                for fc in range(3):
                    ps = ph2ps.tile([128, 512], F32, tag="ps2")
                    for c in range(6):
                        nc.tensor.matmul(
                            ps[:], hT[:, c, 128 * t:128 * t + 128],
                            wvgT[:, c, 512 * fc:512 * fc + 512],
                            start=(c == 0), stop=(c == 5))
                    if with_bias:
                        nc.vector.scalar_tensor_tensor(
                            out=vg_t[:, 512 * fc:512 * fc + 512], in0=ps[:], scalar=1.0,
                            in1=bvgr[:, 512 * fc:512 * fc + 512],
                            op0=OP.mult, op1=OP.add)
                    else:
                        nc.vector.tensor_copy(
                            out=vg_t[:, 512 * fc:512 * fc + 512], in_=ps[:])
                nc.scalar.activation(out=g16[:, t, :], in_=vg_t[:, H:2 * H], func=AF.Gelu)
                for hh in range(NH):
                    nc.vector.tensor_scalar_mul(
                        out=va16[:, t, 65 * hh:65 * hh + 64],
                        in0=vg_t[:, 64 * hh:64 * hh + 64],
                        scalar1=vmask16[:, t:t + 1])
                vav = va16[:, t, :].rearrange("p (h c) -> p h c", h=NH)
                nc.vector.tensor_copy(
                    out=vav[:, :, 64],
                    in_=vmask16[:, t:t + 1].to_broadcast((128, NH)))
            # 2c: QK (feature-major) — emitted last so phase 3 deps are fresh
            for f in range(12):
                for nh in range(2):
                    ps = ph2ps.tile([128, 512], F32, tag="ps2")
                    for c in range(6):
                        nc.tensor.matmul(
                            ps[:], wqkT[:, c, 128 * f:128 * f + 128],
                            hT[:, c, 512 * nh:512 * nh + 512],
                            start=(c == 0), stop=(c == 5))
                    if with_bias:
                        nc.scalar.activation(
                            out=qk16[:, f, 512 * nh:512 * nh + 512], in_=ps[:],
                            func=AF.Identity, bias=bqkc[:, f:f + 1],
                            scale=SCALE if f < 6 else 1.0)
                    else:
                        nc.scalar.activation(
                            out=qk16[:, f, 512 * nh:512 * nh + 512], in_=ps[:],
                            func=AF.Copy, bias=0.0,
                            scale=SCALE if f < 6 else 1.0)

            # 2a: pos projection (needs only wqkT/relT; runs during LN1).
            # evictions on ACT in the no-bias case (DVE is busy with LN1)
            for fc in range(3):
                ps = ph2ps.tile([128, 512], F32, tag="ps2")
                for c in range(6):
                    nc.tensor.matmul(
                        ps[:NB, :], relT[:, c, :], wqkT[:, c, 512 * fc:512 * fc + 512],
                        start=(c == 0), stop=(c == 5))
                if fc == 0:
                    segs = [(0, 512, SCALE)]
                elif fc == 1:
                    segs = [(0, 256, SCALE), (256, 512, 1.0)]
                else:
                    segs = [(0, 512, 1.0)]
                for (a, b_, sc) in segs:
                    if with_bias:
                        nc.vector.scalar_tensor_tensor(
                            out=posp[:NB, 512 * fc + a:512 * fc + b_],
                            in0=ps[:NB, a:b_], scalar=float(sc),
                            in1=bqkr[:NB, 512 * fc + a:512 * fc + b_],
                            op0=OP.mult, op1=OP.add)
                    else:
                        nc.scalar.activation(
                            out=posp[:NB, 512 * fc + a:512 * fc + b_],
                            in_=ps[:NB, a:b_], func=AF.Copy, bias=0.0,
                            scale=float(sc))
            # 2a': M matrices (hidden under the tail of LN1)
            for p in range(6):
                for half in range(2):
                    hh = 2 * p + half
                    r0 = 64 * half
                    for ec in range(2):
                        ps = ph2ps.tile([128, 512], F32, tag="ps2")
                        nc.tensor.matmul(
                            ps[r0:r0 + 64, :],
                            posp[:NB, H + 64 * hh:H + 64 * hh + 64],
                            Ecq[:, 512 * ec:512 * ec + 512],
                            start=True, stop=True, tile_position=(0, r0))
                        nc.scalar.activation(
                            out=Mh[r0:r0 + 64, p, 1 + 512 * ec:1 + 512 * ec + 512],
                            in_=ps[r0:r0 + 64, :], func=AF.Copy)
                        ps2 = ph2ps.tile([128, 512], F32, tag="ps2")
                        nc.tensor.matmul(
                            ps2[r0:r0 + 64, :],
                            posp[:NB, 64 * hh:64 * hh + 64],
                            Eck[:, 512 * ec:512 * ec + 512],
                            start=True, stop=True, tile_position=(0, r0))
                        nc.vector.tensor_copy(
                            out=Mq[r0:r0 + 64, p, 1 + 512 * ec:1 + 512 * ec + 512],
                            in_=ps2[r0:r0 + 64, :])
        wpool.__exit__(None, None, None)   # free wqkT/wvgT/relT/hT

        # --- phase 3: attention, software-pipelined over (b, head), with
        # phase 4 for batch 0 interleaved into the tail. ---
        iters = [(b, hh) for b in range(BL) for hh in range(NH)]
        NIT = len(iters)
        LOOKAHEAD = 2

        ph3ctx = contextlib.ExitStack()
        psA = ph3ctx.enter_context(tc.tile_pool(name="psA", bufs=2, space="PSUM"))
        psS = ph3ctx.enter_context(tc.tile_pool(name="psS", bufs=2, space="PSUM"))
        psP = ph3ctx.enter_context(tc.tile_pool(name="psP", bufs=2, space="PSUM"))
        shp = ph3ctx.enter_context(tc.tile_pool(name="shear", bufs=8))
        etp = ph3ctx.enter_context(tc.tile_pool(name="etp", bufs=8))

        live = {}

        def emit_A(i):
            b, hh = iters[i]
            p, half = hh // 2, hh % 2
            r0 = 64 * half
            tok0 = 512 * b
            cqsh, cksh = [], []
            for t in range(4):
                ws = 384 - 128 * t
                lq = qk16[r0:r0 + 64, p, tok0 + 128 * t:tok0 + 128 * t + 128]
                lk = qk16[r0:r0 + 64, 6 + p, tok0 + 128 * t:tok0 + 128 * t + 128]
                # single 2-bank PSUM tile per direction; one eviction each
                pq = psA.tile([128, WIN], F32, tag="pexp")
                nc.tensor.matmul(pq[:, :512], lq, Mh[r0:r0 + 64, p, ws:ws + 512],
                                 start=True, stop=True)
                nc.tensor.matmul(pq[:, 512:], lq, Mh[r0:r0 + 64, p, ws + 512:ws + 640],
                                 start=True, stop=True)
                wq = shp.tile([128, WIN], F32, tag="wqf", bufs=6)
                if t < 2:
                    nc.scalar.activation(out=wq[:], in_=pq[:], func=AF.Copy)
                else:
                    nc.vector.tensor_copy(out=wq[:], in_=pq[:])
                pk = psA.tile([128, WIN], F32, tag="pexp")
                nc.tensor.matmul(pk[:, :512], lk, Mq[r0:r0 + 64, p, ws:ws + 512],
                                 start=True, stop=True)
                nc.tensor.matmul(pk[:, 512:], lk, Mq[r0:r0 + 64, p, ws + 512:ws + 640],
                                 start=True, stop=True)
                wk = shp.tile([128, WIN], F16, tag="wkf", bufs=6)
                if t < 2:
                    nc.vector.tensor_copy(out=wk[:], in_=pk[:])
                else:
                    nc.scalar.activation(out=wk[:], in_=pk[:], func=AF.Copy)
                cq = shp.tile([128, 512], F32, tag="cqsh", name=f"cqsh{i}_{t}")
                nc.sync.dma_start(out=cq[:], in_=_shear_ap(wq[:], 512))
                cqsh.append(cq)
                ck = shp.tile([128, 512], F16, tag="cksh", name=f"cksh{i}_{t}")
                nc.sync.dma_start(out=ck[:], in_=_shear_ap(wk[:], 512))
                cksh.append(ck)
            live[i] = dict(cqsh=cqsh, cksh=cksh)

        def emit_scores(i):
            b, hh = iters[i]
            p, half = hh // 2, hh % 2
            r0 = 64 * half
            tok0 = 512 * b
            cqsh = live[i]["cqsh"]
            cksh = live[i]["cksh"]
            et = []
            for u in range(4):
                sc = psS.tile([128, 512], F32, tag="sc")
                for t in range(4):
                    nc.tensor.matmul(
                        sc[:, 128 * t:128 * t + 128],
                        cqsh[t][:, 128 * u:128 * u + 128], ident32[:],
                        is_transpose=True, start=(t == 0), stop=False)
                nc.tensor.matmul(
                    sc[:],
                    qk16[r0:r0 + 64, 6 + p, tok0 + 128 * u:tok0 + 128 * u + 128],
                    qk16[r0:r0 + 64, p, tok0:tok0 + 512],
                    start=False, stop=False)
                nc.tensor.matmul(sc[:], ident16[:], cksh[u][:],
                                 start=False, stop=True)
                e_u = etp.tile([128, 512], F16, tag="et")
                nc.scalar.activation(out=e_u[:], in_=sc[:], func=AF.Exp,
                                     bias=negc_t[:], scale=1.0)
                et.append(e_u)
            live[i]["et"] = et

        def emit_pv(i):
            b, hh = iters[i]
            et = live[i]["et"]
            for t in range(4):
                cps = psP.tile([128, 65], F32, tag="cps")
                for u in range(4):
                    nc.tensor.matmul(
                        cps[:], et[u][:, 128 * t:128 * t + 128],
                        va16[:, 4 * b + u, 65 * hh:65 * hh + 65],
                        start=(u == 0), stop=(u == 3))
                rec = stats.tile([128, 1], F32, tag="rec")
                nc.vector.reciprocal(out=rec[:], in_=cps[:, 64:65])
                nc.vector.tensor_scalar_mul(
                    out=ctx16[:, 4 * b + t, 64 * hh:64 * hh + 64],
                    in0=cps[:, 0:64], scalar1=rec[:])
            del live[i]

        for step in range(NIT + LOOKAHEAD):
            if step >= LOOKAHEAD:
                emit_scores(step - LOOKAHEAD)
            if step < NIT:
                emit_A(step)
            if step >= LOOKAHEAD:
                emit_pv(step - LOOKAHEAD)
        ph3ctx.close()

        # --- phase 4: gate + LN2 + PE transposes + out projection ---
        with tc.tile_pool(name="ph4ps", bufs=3, space="PSUM") as ph4ps, \
             tc.tile_pool(name="ph4tps", bufs=2, space="PSUM") as ph4tps, \
             tc.tile_pool(name="ph4", bufs=2) as ph4:
            for t in range(8):
                cg = ph4.tile([128, H], F16, tag="cg")
                nc.gpsimd.tensor_mul(cg[:], ctx16[:, t, :], g16[:, t, :])
                ln2 = ph4.tile([128, H], F16, tag="ln2")
                layernorm_dve(ln2[:], cg[:])
                for c in range(6):
                    tp = ph4tps.tile([128, 128], F16, tag="tps4")
                    nc.tensor.matmul(tp[:], ln2[:, 128 * c:128 * c + 128],
                                     ident16[:], is_transpose=True)
                    if c % 2 == 0:
                        nc.scalar.activation(
                            out=ln2T[:, c, 128 * t:128 * t + 128],
                            in_=tp[:], func=AF.Copy)
                    else:
                        nc.vector.tensor_copy(
                            out=ln2T[:, c, 128 * t:128 * t + 128], in_=tp[:])
                ot = ph4.tile([128, H], F32, tag="ot")
                for fc, (f0, fw) in enumerate([(0, 512), (512, 256)]):
                    ps = ph4ps.tile([128, 512], F32, tag="osc")
                    for c in range(6):
                        nc.tensor.matmul(
                            ps[:, :fw], ln2T[:, c, 128 * t:128 * t + 128],
                            woutT[:, c, f0:f0 + fw],
                            start=(c == 0), stop=(c == 5))
                    if with_bias:
                        nc.vector.scalar_tensor_tensor(
                            out=ot[:, f0:f0 + fw], in0=ps[:, :fw], scalar=1.0,
                            in1=boutr[:, f0:f0 + fw], op0=OP.mult, op1=OP.add)
                    else:
                        nc.vector.tensor_copy(out=ot[:, f0:f0 + fw], in_=ps[:, :fw])
                nc.sync.dma_start(out=out_d[128 * t:128 * t + 128, :], in_=ot[:])

    return nc


# ---------------------------------------------------------------------------
# host side
# ---------------------------------------------------------------------------
def _host_prep(position_indices, attention_mask):
    pi = np.asarray(position_indices)
    gvec = np.empty(1023, np.int64)
    gvec[511:] = pi[:, 0]
    gvec[:512] = pi[0, ::-1]
    d = np.arange(S)[:, None] - np.arange(S)[None, :]
    assert np.array_equal(gvec[d + 511], pi), "position_indices not Toeplitz"
    e = np.arange(1023)
    E_cq = (np.arange(NB)[:, None] == gvec[1022 - e][None, :]).astype(np.float16)
    E_ck = (np.arange(NB)[:, None] == gvec[e][None, :]).astype(np.float16)
    E_cq = np.concatenate([E_cq, np.zeros((NB, 1), np.float16)], 1)
    E_ck = np.concatenate([E_ck, np.zeros((NB, 1), np.float16)], 1)
    am = np.asarray(attention_mask).reshape(B, S)
    vmask = (~am).astype(np.float32)
    return E_cq, E_ck, vmask


def kernel(hidden_states, relative_embedding, w_qk, b_qk, w_vg, b_vg,
           w_out, b_out, attention_mask, position_indices):
    from concourse.bass_utils import run_bass_kernel_spmd

    hidden_states = np.asarray(hidden_states, dtype=np.float32)
    relative_embedding = np.asarray(relative_embedding, dtype=np.float32)
    w_qk = np.asarray(w_qk, dtype=np.float32)
    w_vg = np.asarray(w_vg, dtype=np.float32)
    w_out = np.asarray(w_out, dtype=np.float32)
    b_qk = np.asarray(b_qk, dtype=np.float32)
    b_vg = np.asarray(b_vg, dtype=np.float32)
    b_out = np.asarray(b_out, dtype=np.float32)

    with_bias = bool(np.any(b_qk) or np.any(b_vg) or np.any(b_out))
    E_cq, E_ck, vmask = _host_prep(position_indices, attention_mask)

    nc = build_module(with_bias)
    common = dict(
        wqkT=np.ascontiguousarray(w_qk.T).astype(np.float16),
        wvgT=np.ascontiguousarray(w_vg.T).astype(np.float16),
        woutT=np.ascontiguousarray(w_out.T).astype(np.float16),
        relT=np.ascontiguousarray(relative_embedding.T).astype(np.float16),
        Ecq=E_cq, Eck=E_ck)
    if with_bias:
        sc_col = np.where(np.arange(12) < 6, SCALE, 1.0).astype(np.float32)
        common["bqkc"] = np.ascontiguousarray(
            b_qk.reshape(12, 128).T * sc_col[None, :])
        sc_row = np.concatenate([np.full(H, SCALE), np.ones(H)]).astype(np.float32)
        common["bqkr"] = (b_qk * sc_row)[None, :].astype(np.float32)
        common["bvgr"] = b_vg[None, :].astype(np.float32)
        common["boutr"] = b_out[None, :].astype(np.float32)

    in_maps = []
    for core in range(NCORES):
        bsel = [BL * core + i for i in range(BL)]
        hid = np.ascontiguousarray(
            hidden_states[:, bsel, :].transpose(1, 0, 2).reshape(T, H))
        vm = np.ascontiguousarray(vmask[bsel].reshape(T, 1))
        in_maps.append(dict(common, hid=hid, vmask=vm))

    res = run_bass_kernel_spmd(nc, in_maps, list(range(NCORES)))
    out = np.empty((S, B, H), np.float32)
    for core in range(NCORES):
        o = res.results[core]["out"].reshape(BL, S, H)
        for i in range(BL):
            out[:, BL * core + i, :] = o[i]
    return out


# revision 32
# speedup vs baseline: 1.6197x; 1.3757x over previous
"""Trainium2 Bass kernel for nn_Bert_44452911514066 (DeBERTa-style disentangled
attention BERT layer), data-parallel over batch across 8 NeuronCores.

kernel(**inputs) takes the FULL inputs (as produced by reference.setup_inputs)
and returns the FULL [S, B, H] output.

Key ideas (final):
  - batch-DP: 2 batches per core, weights/tables replicated.
  - relative-position gather is Toeplitz: per (b,h) bucket values are expanded
    into diagonal space by matmuls, the diagonal shear applied by flat-strided
    SBUF->SBUF DMAs, and the cq tiles folded into the [k,q] score PSUM with PE
    transposes (fp32); the ck tiles enter through an identity matmul.
  - the attention loop is software-pipelined two iterations deep so the PE
    never stalls; split 512/128 expansion PSUM tiles keep evictions
    fine-grained (a merged 2-bank tile measured ~40% slower).
  - softmax without max-subtraction: exp(s - 12) on ScalarE; masking and the
    denominator are folded into an augmented/masked V matrix; division on DVE.
  - token<->feature transposes (hT, ln2T) on the PE (fp16 is_transpose into
    fp16 PSUM + tiny evictions) while the PE is otherwise idle; no DRAM
    roundtrip and no DMA-ring traffic.  All DMAs are issued from the SP
    engine only (the ACT HWDGE queue races with cross-queue dependencies in
    this stack), and inputs are loaded in first-use order so LN1/posp/M-build
    hide under the load latency.
  - phase 2 PE order VG -> QK -> posp -> M keeps the PE stream dense and
    leaves qk16/M freshly written when the attention loop starts.
  - LayerNorm statistics alternate DVE (bn_stats) / ACT (Square+accum_out)
    so neither engine serializes phase 1; normalization runs on ScalarE.
  - fp16 matmul inputs everywhere (full PE rate), fp32 accumulation.
    (fp8 DoubleRow projections were tried: slower on this hw and too lossy.)
"""
import sys
sys.path.insert(0, "/opt/trn_rl_repo")
import math
import functools
import contextlib
import numpy as np

import concourse.bass as bass
import concourse.tile as tile
from concourse import mybir
from concourse.masks import make_identity

H, NH, HD, S, B = 768, 12, 64, 512, 16
NCORES = 8
BL = B // NCORES          # batches per core
T = BL * S                # tokens per core
SCALE = 1.0 / math.sqrt(3 * HD)
EPS = 1e-7
NB = 63                   # relative buckets
WIN = 640                 # expansion window per 128-row tile
CSHIFT = 12.0             # exp shift
F16 = mybir.dt.float16
F8 = mybir.dt.float8e4
DR = mybir.MatmulPerfMode.DoubleRow
W8SCALE = 64.0            # fp8 weight pre-scale (host) / descale (eviction)
F32 = mybir.dt.float32
AF = mybir.ActivationFunctionType
OP = mybir.AluOpType

# ---------------------------------------------------------------------------
# walrus workaround: this container's walrus accepts at most ONE sync wait per
# instruction; split extra waits onto single-wait NoOps.
# ---------------------------------------------------------------------------
from concourse.vector_clock import ScopedClock

_orig_add_instruction = tile.TileContext._add_instruction


def _patched_add_instruction(self, inst):
    si = inst.sync_info
    if si is not None and si.on_wait is not None and len(si.on_wait) > 1:
        waits = list(si.on_wait)
        for i, w in enumerate(waits[:-1]):
            nop = mybir.InstNoOp(name=f"{inst.name}-wsplit{i}", ins=[], outs=[])
            nop.engine = inst.engine
            nop.sync_info = mybir.SyncInfo(on_wait=[w], on_update=[])
            _orig_add_instruction(self, nop)
        inst.sync_info = mybir.SyncInfo(
            on_wait=[waits[-1]], on_update=list(si.on_update or []))
    _orig_add_instruction(self, inst)


def _patched_drain_and_barrier(self, tick_clock, wait_clock):
    nc = self.nc
    probe = nc.sync.nop(nofuse=True)
    wait_clock.add_sem_waits(probe.ins, ScopedClock({None: tick_clock.global_clock}))
    si = probe.ins.sync_info
    waits = list(si.on_wait) if si is not None and si.on_wait else []
    if len(waits) > 1:
        probe.ins.sync_info = mybir.SyncInfo(on_wait=waits[:1], on_update=[])
        for w in waits[1:]:
            n2 = nc.sync.nop(nofuse=True)
            n2.ins.sync_info = mybir.SyncInfo(on_wait=[w], on_update=[])
    nc.sync.drain()
    nc.all_engine_barrier()
    assert self.sems is not None
    popped = nc._tile_sem_poison_stack.pop()
    assert popped is self._sem_poison
    nc.clear_and_free_semaphores(list(self.sems.allocated().values()))
    nc.all_engine_barrier()


tile.TileContext._add_instruction = _patched_add_instruction
tile.TileContext._drain_and_barrier = _patched_drain_and_barrier

MW = 1026                 # M-matrix width: 1024 shifted right by 1 (+pad)


def _shear_ap(t_ap, ncols):
    """out[p, j] = tile_flat[p*(WIN-1) + 128 + j] (per-partition offset 128-p)."""
    return bass.AP(tensor=t_ap.tensor, offset=t_ap.offset + 128,
                   ap=[[WIN - 1, 128], [1, ncols]])


# ---------------------------------------------------------------------------
# device kernel build
# ---------------------------------------------------------------------------
@functools.lru_cache(maxsize=4)
def build_module(with_bias: bool, debug: bool = False):
    nc = bass.Bass()

    WDT = F16
    WDS = 1.0
    hid_d = nc.dram_tensor("hid", [T, H], F32, kind="ExternalInput")
    wqkT_d = nc.dram_tensor("wqkT", [H, 2 * H], WDT, kind="ExternalInput")
    wvgT_d = nc.dram_tensor("wvgT", [H, 2 * H], WDT, kind="ExternalInput")
    woutT_d = nc.dram_tensor("woutT", [H, H], WDT, kind="ExternalInput")
    relT_d = nc.dram_tensor("relT", [H, NB], WDT, kind="ExternalInput")
    Ecq_d = nc.dram_tensor("Ecq", [NB, 1024], F16, kind="ExternalInput")
    Eck_d = nc.dram_tensor("Eck", [NB, 1024], F16, kind="ExternalInput")
    vmask_d = nc.dram_tensor("vmask", [T, 1], F32, kind="ExternalInput")
    if with_bias:
        # host-prepared: bqkc[p, f] = b_qk[128f+p] * (SCALE if f<6 else 1)
        bqkc_d = nc.dram_tensor("bqkc", [128, 12], F32, kind="ExternalInput")
        bqkr_d = nc.dram_tensor("bqkr", [1, 2 * H], F32, kind="ExternalInput")
        bvgr_d = nc.dram_tensor("bvgr", [1, 2 * H], F32, kind="ExternalInput")
        boutr_d = nc.dram_tensor("boutr", [1, H], F32, kind="ExternalInput")
    out_d = nc.dram_tensor("out", [T, H], F32, kind="ExternalOutput")

    with tile.TileContext(nc) as tc, contextlib.ExitStack() as ctx:
        persist = ctx.enter_context(tc.tile_pool(name="persist", bufs=1))
        stats = ctx.enter_context(tc.tile_pool(name="stats", bufs=6))

        # --- constants ---
        ident16 = persist.tile([128, 128], F16, tag="id16")
        make_identity(nc, ident16)
        ident32 = persist.tile([128, 128], F32, tag="id32")
        make_identity(nc, ident32)
        eps_t = persist.tile([128, 1], F32, tag="eps")
        nc.vector.memset(eps_t, EPS)
        negc_t = persist.tile([128, 1], F32, tag="negc")
        nc.vector.memset(negc_t, -CSHIFT)
        ln2T = persist.tile([128, 6, T], WDT, tag="ln2T")

        # --- persistent activations/tables ---
        qk16 = persist.tile([128, 12, T], F16, tag="qk16")
        g16 = persist.tile([128, 8, H], F16, tag="g16")
        va16 = persist.tile([128, 8, NH * 65], F16, tag="va16")
        ctx16 = persist.tile([128, 8, H], F16, tag="ctx16")
        Mh = persist.tile([128, 6, MW], F16, tag="Mh")
        Mq = persist.tile([128, 6, MW], F16, tag="Mq")
        nc.vector.memset(Mh[:, :, 0:1], 0.0)
        nc.vector.memset(Mq[:, :, 0:1], 0.0)
        posp = persist.tile([64, 2 * H], F16, tag="posp")
        Ecq = persist.tile([NB, 1024], F16, tag="Ecq")
        Eck = persist.tile([NB, 1024], F16, tag="Eck")
        vmask16 = persist.tile([128, 8], F32, tag="vm")
        woutT = persist.tile([128, 6, H], WDT, tag="woutT")

        # weights needed only through phase 2 live in a scoped pool
        wpool = tc.tile_pool(name="wpool", bufs=1)
        wp = wpool.__enter__()
        wqkT = wp.tile([128, 6, 2 * H], WDT, tag="wqkT")
        wvgT = wp.tile([128, 6, 2 * H], WDT, tag="wvgT")
        relT = wp.tile([128, 6, NB], WDT, tag="relT")
        hT = wp.tile([128, 6, T], WDT, tag="hT")

        # --- LayerNorm helpers ---
        def ln_finish(out16, xin, mean, var):
            rstd = stats.tile([128, 1], F32, tag="rstd")
            nc.scalar.activation(out=rstd[:], in_=var, func=AF.Sqrt,
                                 bias=eps_t[:], scale=1.0)
            nc.vector.reciprocal(out=rstd[:], in_=rstd[:])
            negmr = stats.tile([128, 1], F32, tag="negmr")
            nc.vector.scalar_tensor_tensor(
                out=negmr[:], in0=mean, scalar=-1.0, in1=rstd[:],
                op0=OP.mult, op1=OP.mult)
            nc.scalar.activation(out=out16, in_=xin, func=AF.Identity,
                                 bias=negmr[:], scale=rstd[:])

        def layernorm_dve(out16, xin):
            st = stats.tile([128, 3, 6], F32, tag="bnst")
            for sg in range(3):
                nc.vector.bn_stats(out=st[:, sg, :],
                                   in_=xin[:, 256 * sg:256 * sg + 256])
            mv = stats.tile([128, 2], F32, tag="bnmv")
            nc.vector.bn_aggr(out=mv[:], in_=st[:])
            ln_finish(out16, xin, mv[:, 0:1], mv[:, 1:2])

        def layernorm_actb(out16, xin, sqpool):
            # ACT computes E[x^2] via Square+accum; DVE sums x; Pool smalls
            sq = sqpool.tile([128, H], F32, tag="sq", bufs=2)
            s1 = stats.tile([128, 1], F32, tag="ps1")
            s2 = stats.tile([128, 1], F32, tag="ps2t")
            nc.scalar.activation(out=sq[:], in_=xin, func=AF.Square,
                                 accum_out=s2[:])
            nc.vector.tensor_reduce(out=s1[:], in_=xin,
                                    axis=mybir.AxisListType.X, op=OP.add)
            mean = stats.tile([128, 1], F32, tag="pmean")
            nc.vector.tensor_scalar_mul(out=mean[:], in0=s1[:], scalar1=1.0 / H)
            m2 = stats.tile([128, 1], F32, tag="pm2")
            nc.gpsimd.tensor_mul(m2[:], mean[:], mean[:])
            var = stats.tile([128, 1], F32, tag="pvar")
            nc.vector.scalar_tensor_tensor(
                out=var[:], in0=s2[:], scalar=1.0 / H, in1=m2[:],
                op0=OP.mult, op1=OP.subtract)
            ln_finish(out16, xin, mean[:], var[:])

        # --- phase 0 + 1: input DMAs ordered by first use; LN1; XBAR to hT.
        # Single HWDGE ring (SP) — the ACT ring has broken cross-queue
        # dependency ordering in this stack. ---
        with tc.tile_pool(name="ph1", bufs=2) as ph1:
            xts = []
            for t in range(8):
                xt = ph1.tile([128, H], F32, tag="x", name=f"x{t}", bufs=8)
                nc.sync.dma_start(out=xt[:], in_=hid_d[128 * t:128 * t + 128, :])
                xts.append(xt)
            for c in range(6):
                nc.sync.dma_start(out=wvgT[:, c, :],
                                  in_=wvgT_d[128 * c:128 * c + 128, :])
            for c in range(6):
                nc.sync.dma_start(out=wqkT[:, c, :],
                                  in_=wqkT_d[128 * c:128 * c + 128, :])
            for c in range(6):
                nc.sync.dma_start(out=relT[:, c, :],
                                  in_=relT_d[128 * c:128 * c + 128, :])
            nc.sync.dma_start(out=Ecq[:], in_=Ecq_d[:])
            nc.sync.dma_start(out=Eck[:], in_=Eck_d[:])
            nc.sync.dma_start(
                out=vmask16[:],
                in_=vmask_d[:].rearrange("(t p) one -> p (t one)", p=128))
            if with_bias:
                bqkc = persist.tile([128, 12], F32, tag="bqkc")
                nc.sync.dma_start(out=bqkc[:], in_=bqkc_d[:])
                bqkr = persist.tile([64, 2 * H], F32, tag="bqkr")
                nc.sync.dma_start(
                    out=bqkr[:],
                    in_=bass.AP(tensor=bqkr_d, offset=0, ap=[[0, 64], [1, 2 * H]]))
                bvgr = persist.tile([128, 2 * H], F32, tag="bvgr")
                nc.sync.dma_start(
                    out=bvgr[:],
                    in_=bass.AP(tensor=bvgr_d, offset=0, ap=[[0, 128], [1, 2 * H]]))
                boutr = persist.tile([128, H], F32, tag="boutr")
                nc.sync.dma_start(
                    out=boutr[:],
                    in_=bass.AP(tensor=boutr_d, offset=0, ap=[[0, 128], [1, H]]))
            for c in range(6):
                nc.sync.dma_start(out=woutT[:, c, :],
                                  in_=woutT_d[128 * c:128 * c + 128, :])

            with tc.tile_pool(name="hps", bufs=3, space="PSUM") as hps:
                for t in range(8):
                    h16 = ph1.tile([128, H], F16, tag="h16", bufs=3)
                    if t % 2 == 0:
                        layernorm_dve(h16[:], xts[t][:])
                    else:
                        layernorm_actb(h16[:], xts[t][:], ph1)
                    for c in range(6):
                        tp = hps.tile([128, 128], F16, tag="tps")
                        nc.tensor.matmul(tp[:], h16[:, 128 * c:128 * c + 128],
                                         ident16[:], is_transpose=True)
                        if c % 2 == 0:
                            nc.scalar.activation(
                                out=hT[:, c, 128 * t:128 * t + 128],
                                in_=tp[:], func=AF.Copy)
                        else:
                            nc.vector.tensor_copy(
                                out=hT[:, c, 128 * t:128 * t + 128], in_=tp[:])


        def proj_chain(ps_ap, lhs_fn, rhs_fn):
            """contraction over the 6 128-row chunks: fp16 6x, or fp8 3x DoubleRow"""
            for c in range(6):
                nc.tensor.matmul(ps_ap, lhs_fn(c, 1), rhs_fn(c, 1),
                                 start=(c == 0), stop=(c == 5))

        # --- phase 2: projections ---
        with tc.tile_pool(name="ph2ps", bufs=4, space="PSUM") as ph2ps, \
             tc.tile_pool(name="ph2", bufs=3) as ph2:
            # 2b: VG (token-major) + gelu + masked/augmented V
            for t in range(8):
                vg_t = ph2.tile([128, 2 * H], F16, tag="vg")
                for fc in range(3):
                    ps = ph2ps.tile([128, 512], F32, tag="ps2")
                    proj_chain(ps[:],
                               lambda c, w, t=t: hT[:, c:c + w, 128 * t:128 * t + 128],
                               lambda c, w, fc=fc: wvgT[:, c:c + w, 512 * fc:512 * fc + 512])
                    if with_bias:
                        nc.vector.scalar_tensor_tensor(
                            out=vg_t[:, 512 * fc:512 * fc + 512], in0=ps[:], scalar=1.0,
                            in1=bvgr[:, 512 * fc:512 * fc + 512],
                            op0=OP.mult, op1=OP.add)
                    else:
                        nc.vector.tensor_scalar_mul(
                            out=vg_t[:, 512 * fc:512 * fc + 512], in0=ps[:],
                            scalar1=WDS)
                nc.scalar.activation(out=g16[:, t, :], in_=vg_t[:, H:2 * H], func=AF.Gelu)
                for hh in range(NH):
                    nc.vector.tensor_scalar_mul(
                        out=va16[:, t, 65 * hh:65 * hh + 64],
                        in0=vg_t[:, 64 * hh:64 * hh + 64],
                        scalar1=vmask16[:, t:t + 1])
                vav = va16[:, t, :].rearrange("p (h c) -> p h c", h=NH)
                nc.vector.tensor_copy(
                    out=vav[:, :, 64],
                    in_=vmask16[:, t:t + 1].to_broadcast((128, NH)))
            # 2c: QK (feature-major) — emitted last so phase 3 deps are fresh
            for f in range(12):
                for nh in range(2):
                    ps = ph2ps.tile([128, 512], F32, tag="ps2")
                    proj_chain(ps[:],
                               lambda c, w, f=f: wqkT[:, c:c + w, 128 * f:128 * f + 128],
                               lambda c, w, nh=nh: hT[:, c:c + w, 512 * nh:512 * nh + 512])
                    if with_bias:
                        nc.scalar.activation(
                            out=qk16[:, f, 512 * nh:512 * nh + 512], in_=ps[:],
                            func=AF.Identity, bias=bqkc[:, f:f + 1],
                            scale=SCALE if f < 6 else 1.0)
                    else:
                        nc.scalar.activation(
                            out=qk16[:, f, 512 * nh:512 * nh + 512], in_=ps[:],
                            func=AF.Copy, bias=0.0,
                            scale=(SCALE if f < 6 else 1.0) * WDS)

            # 2a: pos projection (needs only wqkT/relT; runs during LN1).
            # evictions on ACT in the no-bias case (DVE is busy with LN1)
            for fc in range(3):
                ps = ph2ps.tile([128, 512], F32, tag="ps2")
                for c in range(6):
                    nc.tensor.matmul(
                        ps[:NB, :], relT[:, c, :],
                        wqkT[:, c, 512 * fc:512 * fc + 512],
                        start=(c == 0), stop=(c == 5))
                if fc == 0:
                    segs = [(0, 512, SCALE)]
                elif fc == 1:
                    segs = [(0, 256, SCALE), (256, 512, 1.0)]
                else:
                    segs = [(0, 512, 1.0)]
                for (a, b_, sc) in segs:
                    if with_bias:
                        nc.vector.scalar_tensor_tensor(
                            out=posp[:NB, 512 * fc + a:512 * fc + b_],
                            in0=ps[:NB, a:b_], scalar=float(sc),
                            in1=bqkr[:NB, 512 * fc + a:512 * fc + b_],
                            op0=OP.mult, op1=OP.add)
                    else:
                        nc.scalar.activation(
                            out=posp[:NB, 512 * fc + a:512 * fc + b_],
                            in_=ps[:NB, a:b_], func=AF.Copy, bias=0.0,
                            scale=float(sc) * WDS)
            # 2a': M matrices (hidden under the tail of LN1)
            for p in range(6):
                for half in range(2):
                    hh = 2 * p + half
                    r0 = 64 * half
                    for ec in range(2):
                        ps = ph2ps.tile([128, 512], F32, tag="ps2")
                        nc.tensor.matmul(
                            ps[r0:r0 + 64, :],
                            posp[:NB, H + 64 * hh:H + 64 * hh + 64],
                            Ecq[:, 512 * ec:512 * ec + 512],
                            start=True, stop=True, tile_position=(0, r0))
                        nc.scalar.activation(
                            out=Mh[r0:r0 + 64, p, 1 + 512 * ec:1 + 512 * ec + 512],
                            in_=ps[r0:r0 + 64, :], func=AF.Copy)
                        ps2 = ph2ps.tile([128, 512], F32, tag="ps2")
                        nc.tensor.matmul(
                            ps2[r0:r0 + 64, :],
                            posp[:NB, 64 * hh:64 * hh + 64],
                            Eck[:, 512 * ec:512 * ec + 512],
                            start=True, stop=True, tile_position=(0, r0))
                        nc.vector.tensor_copy(
                            out=Mq[r0:r0 + 64, p, 1 + 512 * ec:1 + 512 * ec + 512],
                            in_=ps2[r0:r0 + 64, :])
        wpool.__exit__(None, None, None)   # free wqkT/wvgT/relT/hT

        # --- phase 3: attention, software-pipelined over (b, head), with
        # phase 4 for batch 0 interleaved into the tail. ---
        iters = [(b, hh) for b in range(BL) for hh in range(NH)]
        NIT = len(iters)
        LOOKAHEAD = 2

        ph3ctx = contextlib.ExitStack()
        psA = ph3ctx.enter_context(tc.tile_pool(name="psA", bufs=2, space="PSUM"))
        psS = ph3ctx.enter_context(tc.tile_pool(name="psS", bufs=2, space="PSUM"))
        psP = ph3ctx.enter_context(tc.tile_pool(name="psP", bufs=2, space="PSUM"))
        shp = ph3ctx.enter_context(tc.tile_pool(name="shear", bufs=8))
        etp = ph3ctx.enter_context(tc.tile_pool(name="etp", bufs=8))

        live = {}

        def emit_A(i):
            b, hh = iters[i]
            p, half = hh // 2, hh % 2
            r0 = 64 * half
            tok0 = 512 * b
            cqsh, cksh = [], []
            for t in range(4):
                ws = 384 - 128 * t
                lq = qk16[r0:r0 + 64, p, tok0 + 128 * t:tok0 + 128 * t + 128]
                lk = qk16[r0:r0 + 64, 6 + p, tok0 + 128 * t:tok0 + 128 * t + 128]
                pa = psA.tile([128, 512], F32, tag="pa")
                pb = psA.tile([128, 128], F32, tag="pb")
                nc.tensor.matmul(pa[:], lq, Mh[r0:r0 + 64, p, ws:ws + 512],
                                 start=True, stop=True)
                nc.tensor.matmul(pb[:], lq, Mh[r0:r0 + 64, p, ws + 512:ws + 640],
                                 start=True, stop=True)
                wq = shp.tile([128, WIN], F32, tag="wqf", bufs=6)
                nc.scalar.activation(out=wq[:, :512], in_=pa[:], func=AF.Copy)
                nc.vector.tensor_copy(out=wq[:, 512:], in_=pb[:])
                pa2 = psA.tile([128, 512], F32, tag="pa")
                pb2 = psA.tile([128, 128], F32, tag="pb")
                nc.tensor.matmul(pa2[:], lk, Mq[r0:r0 + 64, p, ws:ws + 512],
                                 start=True, stop=True)
                nc.tensor.matmul(pb2[:], lk, Mq[r0:r0 + 64, p, ws + 512:ws + 640],
                                 start=True, stop=True)
                wk = shp.tile([128, WIN], F16, tag="wkf", bufs=6)
                nc.vector.tensor_copy(out=wk[:, :512], in_=pa2[:])
                nc.vector.tensor_copy(out=wk[:, 512:], in_=pb2[:])
                cq = shp.tile([128, 512], F32, tag="cqsh", name=f"cqsh{i}_{t}")
                nc.sync.dma_start(out=cq[:], in_=_shear_ap(wq[:], 512))
                cqsh.append(cq)
                ck = shp.tile([128, 512], F16, tag="cksh", name=f"cksh{i}_{t}")
                nc.sync.dma_start(out=ck[:], in_=_shear_ap(wk[:], 512))
                cksh.append(ck)
            live[i] = dict(cqsh=cqsh, cksh=cksh)

        def emit_scores(i):
            b, hh = iters[i]
            p, half = hh // 2, hh % 2
            r0 = 64 * half
            tok0 = 512 * b
            cqsh = live[i]["cqsh"]
            cksh = live[i]["cksh"]
            et = []
            for u in range(4):
                sc = psS.tile([128, 512], F32, tag="sc")
                for t in range(4):
                    nc.tensor.matmul(
                        sc[:, 128 * t:128 * t + 128],
                        cqsh[t][:, 128 * u:128 * u + 128], ident32[:],
                        is_transpose=True, start=(t == 0), stop=False)
                nc.tensor.matmul(
                    sc[:],
                    qk16[r0:r0 + 64, 6 + p, tok0 + 128 * u:tok0 + 128 * u + 128],
                    qk16[r0:r0 + 64, p, tok0:tok0 + 512],
                    start=False, stop=False)
                nc.tensor.matmul(sc[:], ident16[:], cksh[u][:],
                                 start=False, stop=True)
                e_u = etp.tile([128, 512], F16, tag="et")
                nc.scalar.activation(out=e_u[:], in_=sc[:], func=AF.Exp,
                                     bias=negc_t[:], scale=1.0)
                et.append(e_u)
            live[i]["et"] = et

        def emit_pv(i):
            b, hh = iters[i]
            et = live[i]["et"]
            for t in range(4):
                cps = psP.tile([128, 65], F32, tag="cps")
                for u in range(4):
                    nc.tensor.matmul(
                        cps[:], et[u][:, 128 * t:128 * t + 128],
                        va16[:, 4 * b + u, 65 * hh:65 * hh + 65],
                        start=(u == 0), stop=(u == 3))
                rec = stats.tile([128, 1], F32, tag="rec")
                nc.vector.reciprocal(out=rec[:], in_=cps[:, 64:65])
                nc.vector.tensor_scalar_mul(
                    out=ctx16[:, 4 * b + t, 64 * hh:64 * hh + 64],
                    in0=cps[:, 0:64], scalar1=rec[:])
            del live[i]

        for step in range(NIT + LOOKAHEAD):
            if step >= LOOKAHEAD:
                emit_scores(step - LOOKAHEAD)
            if step < NIT:
                emit_A(step)
            if step >= LOOKAHEAD:
                emit_pv(step - LOOKAHEAD)
        ph3ctx.close()

        # --- phase 4: gate + LN2 + PE transposes + out projection ---
        with tc.tile_pool(name="ph4ps", bufs=3, space="PSUM") as ph4ps, \
             tc.tile_pool(name="ph4tps", bufs=2, space="PSUM") as ph4tps, \
             tc.tile_pool(name="ph4", bufs=2) as ph4:
            for t in range(8):
                cg = ph4.tile([128, H], F16, tag="cg")
                nc.gpsimd.tensor_mul(cg[:], ctx16[:, t, :], g16[:, t, :])
                ln2 = ph4.tile([128, H], F16, tag="ln2")
                layernorm_dve(ln2[:], cg[:])
                for c in range(6):
                    tp = ph4tps.tile([128, 128], F16, tag="tps4")
                    nc.tensor.matmul(tp[:], ln2[:, 128 * c:128 * c + 128],
                                     ident16[:], is_transpose=True)
                    if c % 2 == 0:
                        nc.scalar.activation(
                            out=ln2T[:, c, 128 * t:128 * t + 128],
                            in_=tp[:], func=AF.Copy)
                    else:
                        nc.vector.tensor_copy(
                            out=ln2T[:, c, 128 * t:128 * t + 128], in_=tp[:])
                ot = ph4.tile([128, H], F32, tag="ot")
                for fc, (f0, fw) in enumerate([(0, 512), (512, 256)]):
                    ps = ph4ps.tile([128, 512], F32, tag="osc")
                    proj_chain(ps[:, :fw],
                               lambda c, w, t=t: ln2T[:, c:c + w, 128 * t:128 * t + 128],
                               lambda c, w, f0=f0, fw=fw: woutT[:, c:c + w, f0:f0 + fw])
                    if with_bias:
                        nc.vector.scalar_tensor_tensor(
                            out=ot[:, f0:f0 + fw], in0=ps[:, :fw], scalar=1.0,
                            in1=boutr[:, f0:f0 + fw], op0=OP.mult, op1=OP.add)
                    else:
                        nc.vector.tensor_scalar_mul(out=ot[:, f0:f0 + fw],
                                                    in0=ps[:, :fw], scalar1=WDS)
                nc.sync.dma_start(out=out_d[128 * t:128 * t + 128, :], in_=ot[:])

    return nc


# ---------------------------------------------------------------------------
# host side
# ---------------------------------------------------------------------------
def _host_prep(position_indices, attention_mask):
    pi = np.asarray(position_indices)
    gvec = np.empty(1023, np.int64)
    gvec[511:] = pi[:, 0]
    gvec[:512] = pi[0, ::-1]
    d = np.arange(S)[:, None] - np.arange(S)[None, :]
    assert np.array_equal(gvec[d + 511], pi), "position_indices not Toeplitz"
    e = np.arange(1023)
    E_cq = (np.arange(NB)[:, None] == gvec[1022 - e][None, :]).astype(np.float16)
    E_ck = (np.arange(NB)[:, None] == gvec[e][None, :]).astype(np.float16)
    E_cq = np.concatenate([E_cq, np.zeros((NB, 1), np.float16)], 1)
    E_ck = np.concatenate([E_ck, np.zeros((NB, 1), np.float16)], 1)
    am = np.asarray(attention_mask).reshape(B, S)
    vmask = (~am).astype(np.float32)
    return E_cq, E_ck, vmask


def kernel(hidden_states, relative_embedding, w_qk, b_qk, w_vg, b_vg,
           w_out, b_out, attention_mask, position_indices):
    from concourse.bass_utils import run_bass_kernel_spmd

    hidden_states = np.asarray(hidden_states, dtype=np.float32)
    relative_embedding = np.asarray(relative_embedding, dtype=np.float32)
    w_qk = np.asarray(w_qk, dtype=np.float32)
    w_vg = np.asarray(w_vg, dtype=np.float32)
    w_out = np.asarray(w_out, dtype=np.float32)
    b_qk = np.asarray(b_qk, dtype=np.float32)
    b_vg = np.asarray(b_vg, dtype=np.float32)
    b_out = np.asarray(b_out, dtype=np.float32)

    with_bias = bool(np.any(b_qk) or np.any(b_vg) or np.any(b_out))
    E_cq, E_ck, vmask = _host_prep(position_indices, attention_mask)

    nc = build_module(with_bias)
    common = dict(
        wqkT=np.ascontiguousarray(w_qk.T).astype(np.float16),
        wvgT=np.ascontiguousarray(w_vg.T).astype(np.float16),
        woutT=np.ascontiguousarray(w_out.T).astype(np.float16),
        relT=np.ascontiguousarray(relative_embedding.T).astype(np.float16),
        Ecq=E_cq, Eck=E_ck)
    if with_bias:
        sc_col = np.where(np.arange(12) < 6, SCALE, 1.0).astype(np.float32)
        common["bqkc"] = np.ascontiguousarray(
            b_qk.reshape(12, 128).T * sc_col[None, :])
        sc_row = np.concatenate([np.full(H, SCALE), np.ones(H)]).astype(np.float32)
        common["bqkr"] = (b_qk * sc_row)[None, :].astype(np.float32)
        common["bvgr"] = b_vg[None, :].astype(np.float32)
        common["boutr"] = b_out[None, :].astype(np.float32)

    in_maps = []
    for core in range(NCORES):
        bsel = [BL * core + i for i in range(BL)]
        hid = np.ascontiguousarray(
            hidden_states[:, bsel, :].transpose(1, 0, 2).reshape(T, H))
        vm = np.ascontiguousarray(vmask[bsel].reshape(T, 1))
        in_maps.append(dict(common, hid=hid, vmask=vm))

    res = run_bass_kernel_spmd(nc, in_maps, list(range(NCORES)))
    out = np.empty((S, B, H), np.float32)
    for core in range(NCORES):
        o = res.results[core]["out"].reshape(BL, S, H)
        for i in range(BL):
            out[:, BL * core + i, :] = o[i]
    return out
